# revision 11
# baseline (speedup 1.0000x reference)
"""Mixtral-style MoE (top-2 of 8 experts) for 8 trn2 NeuronCores.

Strategy: expert-parallel with host-side dispatch.
  - Host computes the (tiny) router: logits = hidden @ gate_w, softmax,
    top-2, renormalize. This decides the sharding: tokens routed to
    expert e are gathered and shipped to core e.
  - Core e runs the expert FFN on its gathered tokens:
        yT = scale * (down_w.T @ (silu(gate_w.T @ x) * (up_w.T @ x)))
    as three fp32r (TF32-like, 1 cyc/row) matmul phases, all in a
    transposed [feature, token] layout so no on-chip transposes are
    needed.
  - Host scatter-adds the per-expert outputs back into [T, H].

All shapes hardcoded per the problem spec:
  B=2, S=2048, H=1024, F=2048, E=8, TOP_K=2, n_cores=8.
"""

from contextlib import ExitStack

import numpy as np

import concourse.bass as bass
import concourse.mybir as mybir
import concourse.tile as tile
from concourse import bacc
from concourse.bass_utils import run_bass_kernel_spmd

H = 1024
F = 2048
E = 8
TOP_K = 2
P = 128
CHUNK = 384          # token chunk (moving-operand width; fp32r needs >=256)

F32 = mybir.dt.float32
F32R = mybir.dt.float32r

_cache: dict = {}


def build_moe_program(C: int, repeats: int = 1):
    """One expert's FFN on C gathered tokens (same NEFF for all 8 cores).

    DRAM in : xt [H, C] fp32 (gathered tokens, transposed)
              wg [H, F], wu [H, F], wd [F, H] fp32 (this expert's weights)
              scale [1, C] fp32 (renormalized routing weight per slot)
    DRAM out: yt [H, C] fp32 = scale * down(silu(gate(x)) * up(x)).T

    repeats > 1 re-runs the whole body (benchmarking only: the wall-time
    slope over repeats isolates per-iteration HW time from dispatch
    overhead).
    """
    assert C % CHUNK == 0
    assert C <= 1536, (
        f"C={C} exceeds SBUF residency budget (max 1536 tokens/expert)")
    NC_CH = C // CHUNK     # token chunks
    NKH = H // P           # 8  k-chunks over H
    NF = F // P            # 16 f-tiles
    NKF = F // P           # 16 k-chunks over F
    NH = H // P            # 8  h-tiles

    nc = bacc.Bacc("TRN2", target_bir_lowering=False, debug=False, num_devices=E)

    # Declared float32r (same 4-byte layout as fp32): the PE reads fp32r
    # directly and all DMAs stay on the fast non-casting HWDGE path.
    xt = nc.dram_tensor("xt", [H, C], F32R, kind="ExternalInput")
    wg = nc.dram_tensor("wg", [H, F], F32R, kind="ExternalInput")
    wu = nc.dram_tensor("wu", [H, F], F32R, kind="ExternalInput")
    wd = nc.dram_tensor("wd", [F, H], F32R, kind="ExternalInput")
    scale = nc.dram_tensor("scale", [1, C], F32, kind="ExternalInput")
    yt = nc.dram_tensor("yt", [H, C], F32, kind="ExternalOutput")

    # DRAM views grouping the partition-dim into 128-row chunks:
    # wg/wu [H, F] -> [p, k, f] ; wd [F, H] -> [p, k, h]
    wg_v = wg.rearrange("(k p) f -> p k f", p=P)
    wu_v = wu.rearrange("(k p) f -> p k f", p=P)
    wd_v = wd.rearrange("(k p) h -> p k h", p=P)

    with tile.TileContext(nc) as tc, ExitStack() as ctx:
        # --- resident pools ---
        xt_pool = ctx.enter_context(tc.tile_pool(name="xt", bufs=1))
        at_pool = ctx.enter_context(tc.tile_pool(name="at", bufs=1))
        sc_pool = ctx.enter_context(tc.tile_pool(name="sc", bufs=1))
        # --- streaming pools ---
        wbufs = 2 if C <= 1200 else 1
        wgu_pool = ctx.enter_context(tc.tile_pool(name="wgu", bufs=wbufs))
        wd_pool = ctx.enter_context(tc.tile_pool(name="wd", bufs=wbufs))
        silu_pool = ctx.enter_context(tc.tile_pool(name="silu", bufs=3))
        yo_pool = ctx.enter_context(tc.tile_pool(name="yo", bufs=3))
        psA = ctx.enter_context(tc.tile_pool(name="psA", bufs=2, space="PSUM"))
        psB = ctx.enter_context(tc.tile_pool(name="psB", bufs=2, space="PSUM"))

        for _rep in range(repeats):
            _build_body(nc, tc, C, NC_CH, NKH, NF, NKF, NH,
                        xt, wg_v, wu_v, wd_v, scale, yt,
                        xt_pool, at_pool, sc_pool, wgu_pool, wd_pool,
                        silu_pool, yo_pool, psA, psB)
    nc.compile()
    return nc


def _build_body(nc, tc, C, NC_CH, NKH, NF, NKF, NH,
                xt, wg_v, wu_v, wd_v, scale, yt,
                xt_pool, at_pool, sc_pool, wgu_pool, wd_pool,
                silu_pool, yo_pool, psA, psB):
    if True:
        # xt resident: 8 tiles [128, C], cast to fp32r on load (gpsimd DMA)
        xt_sb = []
        for k in range(NKH):
            t = xt_pool.tile([P, C], F32R, tag=f"xt{k}", name=f"xt_sb{k}")
            # per-chunk loads so the first matmuls start after ~1/NC_CH of
            # the xt bytes have landed
            for c in range(NC_CH):
                nc.sync.dma_start(t[:, bass.ts(c, CHUNK)],
                                  xt[k * P:(k + 1) * P, bass.ts(c, CHUNK)])
            xt_sb.append(t)

        # scale broadcast to all partitions: [128, C]
        sc_sb = sc_pool.tile([P, C], F32)
        nc.sync.dma_start(sc_sb[:], scale[0:1, :].to_broadcast((P, C)))

        # aT resident: 16 tiles [128, C] fp32r (silu(g)*u, transposed)
        at_sb = [at_pool.tile([P, C], F32R, tag=f"at{f}", name=f"at_sb{f}")
                 for f in range(NF)]

        # ---- Phase A: aT[f][:, c] = silu(gT) * uT ----
        for f in range(NF):
            # weight tiles for this f: [128, NKH*128] with k-chunk blocks
            wgf = wgu_pool.tile([P, NKH * P], F32R, tag="wgf")
            nc.sync.dma_start(wgf[:], wg_v[:, :, f * P:(f + 1) * P])
            wuf = wgu_pool.tile([P, NKH * P], F32R, tag="wuf")
            nc.sync.dma_start(wuf[:], wu_v[:, :, f * P:(f + 1) * P])
            for c in range(NC_CH):
                csl = bass.ts(c, CHUNK)
                pg = psA.tile([P, CHUNK], F32, tag="pg")
                pu = psA.tile([P, CHUNK], F32, tag="pu")
                for k in range(NKH):
                    nc.tensor.matmul(pg[:], wgf[:, k * P:(k + 1) * P],
                                     xt_sb[k][:, csl],
                                     start=(k == 0), stop=(k == NKH - 1))
                for k in range(NKH):
                    nc.tensor.matmul(pu[:], wuf[:, k * P:(k + 1) * P],
                                     xt_sb[k][:, csl],
                                     start=(k == 0), stop=(k == NKH - 1))
                st = silu_pool.tile([P, CHUNK], F32, tag="st")
                nc.scalar.activation(st[:], pg[:],
                                     mybir.ActivationFunctionType.Sigmoid)
                s2 = silu_pool.tile([P, CHUNK], F32, tag="s2")
                nc.vector.tensor_mul(s2[:], st[:], pg[:])
                nc.vector.tensor_mul(at_sb[f][:, csl], s2[:], pu[:])

        # ---- Phase B: yt[h][:, c] = scale * sum_k wd[k,h].T @ aT[k][:, c] ----
        for h in range(NH):
            wdh = wd_pool.tile([P, NKF * P], F32R, tag="wdh")
            nc.sync.dma_start(wdh[:], wd_v[:, :, h * P:(h + 1) * P])
            for c in range(NC_CH):
                csl = bass.ts(c, CHUNK)
                py = psB.tile([P, CHUNK], F32, tag="py")
                for k in range(NKF):
                    nc.tensor.matmul(py[:], wdh[:, k * P:(k + 1) * P],
                                     at_sb[k][:, csl],
                                     start=(k == 0), stop=(k == NKF - 1))
                yo = yo_pool.tile([P, CHUNK], F32)
                nc.vector.tensor_mul(yo[:], py[:], sc_sb[:, csl])
                nc.sync.dma_start(yt[h * P:(h + 1) * P, c * CHUNK:(c + 1) * CHUNK],
                                  yo[:])


def _route(hidden: np.ndarray, gate_w: np.ndarray):
    """Host router: returns (idx [T, K], w [T, K] renormalized fp32)."""
    logits = hidden.astype(np.float32) @ gate_w.astype(np.float32)
    m = logits.max(axis=-1, keepdims=True)
    e = np.exp((logits - m).astype(np.float32))
    p = e / e.sum(axis=-1, keepdims=True)
    # top-2, ties -> lower index (match jax.lax.top_k)
    order = np.argsort(-p, axis=-1, kind="stable")
    idx = order[:, :TOP_K]
    topw = np.take_along_axis(p, idx, axis=-1)
    topw = topw / topw.sum(axis=-1, keepdims=True)
    return idx, topw.astype(np.float32)


def kernel(hidden_states, gate_w, gate_proj_w, up_proj_w, down_proj_w):
    B, S, Hx = hidden_states.shape
    T = B * S
    hidden = np.ascontiguousarray(
        np.asarray(hidden_states, dtype=np.float32).reshape(T, Hx))

    idx, topw = _route(hidden, np.asarray(gate_w))

    # Per-expert token lists
    rows, wts = [], []
    for e in range(E):
        mask = (idx == e)
        r = np.nonzero(mask.any(axis=-1))[0]
        rows.append(r)
        wts.append(topw[r, np.argmax(idx[r] == e, axis=-1)])
    maxn = max(len(r) for r in rows)
    C = max(CHUNK, ((maxn + CHUNK - 1) // CHUNK) * CHUNK)

    if C not in _cache:
        _cache[C] = build_moe_program(C)
    nc = _cache[C]

    gate_proj_w = np.asarray(gate_proj_w, dtype=np.float32)
    up_proj_w = np.asarray(up_proj_w, dtype=np.float32)
    down_proj_w = np.asarray(down_proj_w, dtype=np.float32)

    in_maps = []
    for e in range(E):
        r = rows[e]
        xt = np.zeros((Hx, C), dtype=np.float32)
        xt[:, :len(r)] = hidden[r].T
        sc = np.zeros((1, C), dtype=np.float32)
        sc[0, :len(r)] = wts[e]
        in_maps.append({
            "xt": xt,
            "wg": np.ascontiguousarray(gate_proj_w[e]),
            "wu": np.ascontiguousarray(up_proj_w[e]),
            "wd": np.ascontiguousarray(down_proj_w[e]),
            "scale": sc,
        })

    global _last_in_maps, _last_rows
    _last_in_maps = in_maps
    _last_rows = rows
    res = run_bass_kernel_spmd(nc, in_maps, core_ids=list(range(E)))

    out = np.zeros((T, Hx), dtype=np.float32)
    for e in range(E):
        r = rows[e]
        out[r] += res.results[e]["yt"][:, :len(r)].T
    return out.reshape(B, S, Hx)


# revision 15
# speedup vs baseline: 1.1642x; 1.1642x over previous
"""Mixtral-style MoE (top-2 of 8 experts) for 8 trn2 NeuronCores.

Strategy: expert-parallel with host-side dispatch.
  - Host computes the (tiny) router: logits = hidden @ gate_w, softmax,
    top-2, renormalize. This decides the sharding: tokens routed to
    expert e are gathered and shipped to core e.
  - Core e runs the expert FFN on its gathered tokens:
        yT = scale * (down_w.T @ (silu(gate_w.T @ x) * (up_w.T @ x)))
    as three fp32r (TF32-like, 1 cyc/row) matmul phases, all in a
    transposed [feature, token] layout so no on-chip transposes are
    needed.
  - Host scatter-adds the per-expert outputs back into [T, H].

All shapes hardcoded per the problem spec:
  B=2, S=2048, H=1024, F=2048, E=8, TOP_K=2, n_cores=8.
"""

from contextlib import ExitStack

import numpy as np

import concourse.bass as bass
import concourse.mybir as mybir
import concourse.tile as tile
from concourse import bacc
from concourse.bass_utils import run_bass_kernel_spmd

H = 1024
F = 2048
E = 8
TOP_K = 2
P = 128
MIN_CHUNK = 256      # fp32r needs a moving dim >= 256 for 1 cyc/row
MAX_CHUNK = 512      # one PSUM bank


def token_chunks(C: int):
    """Split C into near-equal chunks, each a multiple of 8 in
    [MIN_CHUNK, MAX_CHUNK].

    fp32r matmuls require an even moving dim per the ISA verifier
    (s3d3_mm_fp32r_restrictions), but HW-probing showed even-but-not-
    multiple-of-8 widths (e.g. 364) crash the device; multiples of 8
    (368/360/344/296) run correctly. So chunks are multiples of 8.
    """
    assert C % 8 == 0
    n = max(1, -(-C // MAX_CHUNK))
    h = C // 8
    sizes = [8 * (h // n + (1 if i < h % n else 0)) for i in range(n)]
    assert sum(sizes) == C and all(MIN_CHUNK <= sz <= MAX_CHUNK for sz in sizes)
    offs = [sum(sizes[:i]) for i in range(n)]
    return list(zip(offs, sizes))

F32 = mybir.dt.float32
F32R = mybir.dt.float32r

_cache: dict = {}


def build_moe_program(C: int, repeats: int = 1):
    """One expert's FFN on C gathered tokens (same NEFF for all 8 cores).

    DRAM in : xt [H, C] fp32 (gathered tokens, transposed)
              wg [H, F], wu [H, F], wd [F, H] fp32 (this expert's weights)
              scale [1, C] fp32 (renormalized routing weight per slot)
    DRAM out: yt [H, C] fp32 = scale * down(silu(gate(x)) * up(x)).T

    repeats > 1 re-runs the whole body (benchmarking only: the wall-time
    slope over repeats isolates per-iteration HW time from dispatch
    overhead).
    """
    assert C >= MIN_CHUNK
    assert C <= 1536, (
        f"C={C} exceeds SBUF residency budget (max 1536 tokens/expert)")
    CH = token_chunks(C)   # [(offset, size)] token chunks
    NKH = H // P           # 8  k-chunks over H
    NF = F // P            # 16 f-tiles
    NKF = F // P           # 16 k-chunks over F
    NH = H // P            # 8  h-tiles

    nc = bacc.Bacc("TRN2", target_bir_lowering=False, debug=False, num_devices=E)

    # Declared float32r (same 4-byte layout as fp32): the PE reads fp32r
    # directly and all DMAs stay on the fast non-casting HWDGE path.
    xt = nc.dram_tensor("xt", [H, C], F32R, kind="ExternalInput")
    wg = nc.dram_tensor("wg", [H, F], F32R, kind="ExternalInput")
    wu = nc.dram_tensor("wu", [H, F], F32R, kind="ExternalInput")
    wd = nc.dram_tensor("wd", [F, H], F32R, kind="ExternalInput")
    scale = nc.dram_tensor("scale", [1, C], F32, kind="ExternalInput")
    yt = nc.dram_tensor("yt", [H, C], F32, kind="ExternalOutput")

    # DRAM views grouping the partition-dim into 128-row chunks:
    # wg/wu [H, F] -> [p, k, f] ; wd [F, H] -> [p, k, h]
    wg_v = wg.rearrange("(k p) f -> p k f", p=P)
    wu_v = wu.rearrange("(k p) f -> p k f", p=P)
    wd_v = wd.rearrange("(k p) h -> p k h", p=P)

    with tile.TileContext(nc) as tc, ExitStack() as ctx:
        # --- resident pools ---
        xt_pool = ctx.enter_context(tc.tile_pool(name="xt", bufs=1))
        at_pool = ctx.enter_context(tc.tile_pool(name="at", bufs=1))
        sc_pool = ctx.enter_context(tc.tile_pool(name="sc", bufs=1))
        # --- streaming pools ---
        wbufs = 2 if C <= 1200 else 1
        wgu_pool = ctx.enter_context(tc.tile_pool(name="wgu", bufs=wbufs))
        wd_pool = ctx.enter_context(tc.tile_pool(name="wd", bufs=wbufs))
        silu_pool = ctx.enter_context(tc.tile_pool(name="silu", bufs=3))
        yo_pool = ctx.enter_context(tc.tile_pool(name="yo", bufs=3))
        psA = ctx.enter_context(tc.tile_pool(name="psA", bufs=2, space="PSUM"))
        psB = ctx.enter_context(tc.tile_pool(name="psB", bufs=2, space="PSUM"))

        for _rep in range(repeats):
            _build_body(nc, tc, C, CH, NKH, NF, NKF, NH,
                        xt, wg_v, wu_v, wd_v, scale, yt,
                        xt_pool, at_pool, sc_pool, wgu_pool, wd_pool,
                        silu_pool, yo_pool, psA, psB)
    nc.compile()
    return nc


def _build_body(nc, tc, C, CH, NKH, NF, NKF, NH,
                xt, wg_v, wu_v, wd_v, scale, yt,
                xt_pool, at_pool, sc_pool, wgu_pool, wd_pool,
                silu_pool, yo_pool, psA, psB):
    if True:
        # xt resident: 8 tiles [128, C] (fp32r view of the fp32 bytes)
        xt_sb = []
        for k in range(NKH):
            t = xt_pool.tile([P, C], F32R, tag=f"xt{k}", name=f"xt_sb{k}")
            # per-chunk loads so the first matmuls start after ~1/len(CH)
            # of the xt bytes have landed
            for off, sz in CH:
                nc.sync.dma_start(t[:, off:off + sz],
                                  xt[k * P:(k + 1) * P, off:off + sz])
            xt_sb.append(t)

        # scale broadcast to all partitions: [128, C]
        sc_sb = sc_pool.tile([P, C], F32)
        nc.sync.dma_start(sc_sb[:], scale[0:1, :].to_broadcast((P, C)))

        # aT resident: 16 tiles [128, C] fp32r (silu(g)*u, transposed)
        at_sb = [at_pool.tile([P, C], F32R, tag=f"at{f}", name=f"at_sb{f}")
                 for f in range(NF)]

        # ---- Phase A: aT[f][:, c] = silu(gT) * uT ----
        for f in range(NF):
            # weight tiles for this f: [128, NKH*128] with k-chunk blocks
            wgf = wgu_pool.tile([P, NKH * P], F32R, tag="wgf")
            nc.sync.dma_start(wgf[:], wg_v[:, :, f * P:(f + 1) * P])
            wuf = wgu_pool.tile([P, NKH * P], F32R, tag="wuf")
            nc.sync.dma_start(wuf[:], wu_v[:, :, f * P:(f + 1) * P])
            for off, sz in CH:
                csl = slice(off, off + sz)
                pg_t = psA.tile([P, MAX_CHUNK], F32, tag="pg")
                pu_t = psA.tile([P, MAX_CHUNK], F32, tag="pu")
                pg, pu = pg_t[:, :sz], pu_t[:, :sz]
                for k in range(NKH):
                    nc.tensor.matmul(pg, wgf[:, k * P:(k + 1) * P],
                                     xt_sb[k][:, csl],
                                     start=(k == 0), stop=(k == NKH - 1))
                for k in range(NKH):
                    nc.tensor.matmul(pu, wuf[:, k * P:(k + 1) * P],
                                     xt_sb[k][:, csl],
                                     start=(k == 0), stop=(k == NKH - 1))
                st_t = silu_pool.tile([P, MAX_CHUNK], F32, tag="st")
                st = st_t[:, :sz]
                nc.scalar.activation(st, pg,
                                     mybir.ActivationFunctionType.Sigmoid)
                s2_t = silu_pool.tile([P, MAX_CHUNK], F32, tag="s2")
                s2 = s2_t[:, :sz]
                nc.vector.tensor_mul(s2, st, pg)
                nc.vector.tensor_mul(at_sb[f][:, csl], s2, pu)

        # ---- Phase B: yt[h][:, c] = scale * sum_k wd[k,h].T @ aT[k][:, c] ----
        for h in range(NH):
            wdh = wd_pool.tile([P, NKF * P], F32R, tag="wdh")
            nc.sync.dma_start(wdh[:], wd_v[:, :, h * P:(h + 1) * P])
            for off, sz in CH:
                csl = slice(off, off + sz)
                py_t = psB.tile([P, MAX_CHUNK], F32, tag="py")
                py = py_t[:, :sz]
                for k in range(NKF):
                    nc.tensor.matmul(py, wdh[:, k * P:(k + 1) * P],
                                     at_sb[k][:, csl],
                                     start=(k == 0), stop=(k == NKF - 1))
                yo_t = yo_pool.tile([P, MAX_CHUNK], F32, tag="yo")
                yo = yo_t[:, :sz]
                nc.vector.tensor_mul(yo, py, sc_sb[:, csl])
                nc.sync.dma_start(yt[h * P:(h + 1) * P, off:off + sz], yo)


def _route(hidden: np.ndarray, gate_w: np.ndarray):
    """Host router: returns (idx [T, K], w [T, K] renormalized fp32)."""
    logits = hidden.astype(np.float32) @ gate_w.astype(np.float32)
    m = logits.max(axis=-1, keepdims=True)
    e = np.exp((logits - m).astype(np.float32))
    p = e / e.sum(axis=-1, keepdims=True)
    # top-2, ties -> lower index (match jax.lax.top_k)
    order = np.argsort(-p, axis=-1, kind="stable")
    idx = order[:, :TOP_K]
    topw = np.take_along_axis(p, idx, axis=-1)
    topw = topw / topw.sum(axis=-1, keepdims=True)
    return idx, topw.astype(np.float32)


def kernel(hidden_states, gate_w, gate_proj_w, up_proj_w, down_proj_w):
    B, S, Hx = hidden_states.shape
    T = B * S
    hidden = np.ascontiguousarray(
        np.asarray(hidden_states, dtype=np.float32).reshape(T, Hx))

    idx, topw = _route(hidden, np.asarray(gate_w))

    # Per-expert token lists
    rows, wts = [], []
    for e in range(E):
        mask = (idx == e)
        r = np.nonzero(mask.any(axis=-1))[0]
        rows.append(r)
        wts.append(topw[r, np.argmax(idx[r] == e, axis=-1)])
    maxn = max(len(r) for r in rows)
    C = max(MIN_CHUNK, ((maxn + 7) // 8) * 8)

    if C not in _cache:
        _cache[C] = build_moe_program(C)
    nc = _cache[C]

    gate_proj_w = np.asarray(gate_proj_w, dtype=np.float32)
    up_proj_w = np.asarray(up_proj_w, dtype=np.float32)
    down_proj_w = np.asarray(down_proj_w, dtype=np.float32)

    in_maps = []
    for e in range(E):
        r = rows[e]
        xt = np.zeros((Hx, C), dtype=np.float32)
        xt[:, :len(r)] = hidden[r].T
        sc = np.zeros((1, C), dtype=np.float32)
        sc[0, :len(r)] = wts[e]
        in_maps.append({
            "xt": xt,
            "wg": np.ascontiguousarray(gate_proj_w[e]),
            "wu": np.ascontiguousarray(up_proj_w[e]),
            "wd": np.ascontiguousarray(down_proj_w[e]),
            "scale": sc,
        })

    global _last_in_maps, _last_rows
    _last_in_maps = in_maps
    _last_rows = rows
    res = run_bass_kernel_spmd(nc, in_maps, core_ids=list(range(E)))

    out = np.zeros((T, Hx), dtype=np.float32)
    for e in range(E):
        r = rows[e]
        out[r] += res.results[e]["yt"][:, :len(r)].T
    return out.reshape(B, S, Hx)



# revision 16
# speedup vs baseline: 1.6635x; 1.4289x over previous
"""Mixtral-style MoE (top-2 of 8 experts) for 8 trn2 NeuronCores.

Strategy: expert-parallel with host-side dispatch.
  - Host computes the (tiny) router: logits = hidden @ gate_w, softmax,
    top-2, renormalize. This decides the sharding: tokens routed to
    expert e are gathered and shipped to core e.
  - Core e runs the expert FFN on its gathered tokens:
        yT = scale * (down_w.T @ (silu(gate_w.T @ x) * (up_w.T @ x)))
    as three fp32r (TF32-like, 1 cyc/row) matmul phases, all in a
    transposed [feature, token] layout so no on-chip transposes are
    needed.
  - Host scatter-adds the per-expert outputs back into [T, H].

All shapes hardcoded per the problem spec:
  B=2, S=2048, H=1024, F=2048, E=8, TOP_K=2, n_cores=8.
"""

from contextlib import ExitStack

import numpy as np

import concourse.bass as bass
import concourse.mybir as mybir
import concourse.tile as tile
from concourse import bacc
from concourse.bass_utils import run_bass_kernel_spmd

H = 1024
F = 2048
E = 8
TOP_K = 2
P = 128
MIN_CHUNK = 256      # fp32r needs a moving dim >= 256 for 1 cyc/row
MAX_CHUNK = 512      # one PSUM bank


def token_chunks(C: int):
    """Split C into near-equal chunks, each a multiple of 8 in
    [MIN_CHUNK, MAX_CHUNK].

    fp32r matmuls require an even moving dim per the ISA verifier
    (s3d3_mm_fp32r_restrictions), but HW-probing showed even-but-not-
    multiple-of-8 widths (e.g. 364) crash the device; multiples of 8
    (368/360/344/296) run correctly. So chunks are multiples of 8.
    """
    assert C % 8 == 0
    n = max(1, -(-C // MAX_CHUNK))
    h = C // 8
    sizes = [8 * (h // n + (1 if i < h % n else 0)) for i in range(n)]
    assert sum(sizes) == C and all(MIN_CHUNK <= sz <= MAX_CHUNK for sz in sizes)
    offs = [sum(sizes[:i]) for i in range(n)]
    return list(zip(offs, sizes))

F32 = mybir.dt.float32
F32R = mybir.dt.float32r

_cache: dict = {}


def build_moe_program(C: int, repeats: int = 1):
    """One expert's FFN on C gathered tokens (same NEFF for all 8 cores).

    DRAM in : xt [H, C] fp32 (gathered tokens, transposed)
              wg [H, F], wu [H, F], wd [F, H] fp32 (this expert's weights)
              scale [1, C] fp32 (renormalized routing weight per slot)
    DRAM out: yt [H, C] fp32 = scale * down(silu(gate(x)) * up(x)).T

    repeats > 1 re-runs the whole body (benchmarking only: the wall-time
    slope over repeats isolates per-iteration HW time from dispatch
    overhead).
    """
    assert C >= MIN_CHUNK
    assert C <= 1536, (
        f"C={C} exceeds SBUF residency budget (max 1536 tokens/expert)")
    CH = token_chunks(C)   # [(offset, size)] token chunks
    NKH = H // P           # 8  k-chunks over H
    NF = F // P            # 16 f-tiles
    NKF = F // P           # 16 k-chunks over F
    NH = H // P            # 8  h-tiles

    nc = bacc.Bacc("TRN2", target_bir_lowering=False, debug=False, num_devices=E)

    # Declared float32r (same 4-byte layout as fp32): the PE reads fp32r
    # directly and all DMAs stay on the fast non-casting HWDGE path.
    xt = nc.dram_tensor("xt", [H, C], F32R, kind="ExternalInput")
    wg = nc.dram_tensor("wg", [H, F], F32R, kind="ExternalInput")
    wu = nc.dram_tensor("wu", [H, F], F32R, kind="ExternalInput")
    wd = nc.dram_tensor("wd", [F, H], F32R, kind="ExternalInput")
    scale = nc.dram_tensor("scale", [1, C], F32, kind="ExternalInput")
    yt = nc.dram_tensor("yt", [H, C], F32, kind="ExternalOutput")

    # DRAM views grouping the partition-dim into 128-row chunks:
    # wg/wu [H, F] -> [p, k, f] ; wd [F, H] -> [p, k, h]
    wg_v = wg.rearrange("(k p) f -> p k f", p=P)
    wu_v = wu.rearrange("(k p) f -> p k f", p=P)
    wd_v = wd.rearrange("(k p) h -> p k h", p=P)

    with tile.TileContext(nc) as tc, ExitStack() as ctx:
        # --- resident pools ---
        xt_pool = ctx.enter_context(tc.tile_pool(name="xt", bufs=1))
        at_pool = ctx.enter_context(tc.tile_pool(name="at", bufs=1))
        sc_pool = ctx.enter_context(tc.tile_pool(name="sc", bufs=1))
        # --- streaming pools ---
        wbufs = 2 if C <= 1200 else 1
        wgu_pool = ctx.enter_context(tc.tile_pool(name="wgu", bufs=wbufs))
        wd_pool = ctx.enter_context(tc.tile_pool(name="wd", bufs=wbufs))
        silu_pool = ctx.enter_context(tc.tile_pool(name="silu", bufs=3))
        yo_pool = ctx.enter_context(tc.tile_pool(name="yo", bufs=3))
        psA = ctx.enter_context(tc.tile_pool(name="psA", bufs=2, space="PSUM"))
        psB = ctx.enter_context(tc.tile_pool(name="psB", bufs=2, space="PSUM"))

        for _rep in range(repeats):
            _build_body(nc, tc, C, CH, NKH, NF, NKF, NH,
                        xt, wg_v, wu_v, wd_v, scale, yt,
                        xt_pool, at_pool, sc_pool, wgu_pool, wd_pool,
                        silu_pool, yo_pool, psA, psB)
    nc.compile()
    return nc


def _build_body(nc, tc, C, CH, NKH, NF, NKF, NH,
                xt, wg_v, wu_v, wd_v, scale, yt,
                xt_pool, at_pool, sc_pool, wgu_pool, wd_pool,
                silu_pool, yo_pool, psA, psB):
    if True:
        # xt resident: 8 tiles [128, C] (fp32r view of the fp32 bytes)
        xt_sb = []
        for k in range(NKH):
            t = xt_pool.tile([P, C], F32R, tag=f"xt{k}", name=f"xt_sb{k}")
            # per-chunk loads so the first matmuls start after ~1/len(CH)
            # of the xt bytes have landed
            for off, sz in CH:
                nc.sync.dma_start(t[:, off:off + sz],
                                  xt[k * P:(k + 1) * P, off:off + sz])
            xt_sb.append(t)

        # aT resident: 16 tiles [128, C] fp32r (silu(g)*u, transposed)
        at_sb = [at_pool.tile([P, C], F32R, tag=f"at{f}", name=f"at_sb{f}")
                 for f in range(NF)]

        # ---- Phase A: aT[f][:, c] = silu(gT) * uT ----
        for f in range(NF):
            # weight tiles for this f: [128, NKH*128] with k-chunk blocks
            wgf = wgu_pool.tile([P, NKH * P], F32R, tag="wgf")
            nc.sync.dma_start(wgf[:], wg_v[:, :, f * P:(f + 1) * P])
            wuf = wgu_pool.tile([P, NKH * P], F32R, tag="wuf")
            nc.sync.dma_start(wuf[:], wu_v[:, :, f * P:(f + 1) * P])
            for off, sz in CH:
                csl = slice(off, off + sz)
                pg_t = psA.tile([P, MAX_CHUNK], F32, tag="pg")
                pu_t = psA.tile([P, MAX_CHUNK], F32, tag="pu")
                pg, pu = pg_t[:, :sz], pu_t[:, :sz]
                for k in range(NKH):
                    nc.tensor.matmul(pg, wgf[:, k * P:(k + 1) * P],
                                     xt_sb[k][:, csl],
                                     start=(k == 0), stop=(k == NKH - 1))
                for k in range(NKH):
                    nc.tensor.matmul(pu, wuf[:, k * P:(k + 1) * P],
                                     xt_sb[k][:, csl],
                                     start=(k == 0), stop=(k == NKH - 1))
                st_t = silu_pool.tile([P, MAX_CHUNK], F32, tag="st")
                st = st_t[:, :sz]
                nc.scalar.activation(st, pg,
                                     mybir.ActivationFunctionType.Sigmoid)
                s2_t = silu_pool.tile([P, MAX_CHUNK], F32, tag="s2")
                s2 = s2_t[:, :sz]
                nc.vector.tensor_mul(s2, st, pg)
                nc.vector.tensor_mul(at_sb[f][:, csl], s2, pu)

        # ---- Phase B: yt[h][:, c] = scale * sum_k wd[k,h].T @ aT[k][:, c] ----
        # scale broadcast to all partitions (emitted here so its DMA does
        # not delay the phase-A weight loads at startup)
        sc_sb = sc_pool.tile([P, C], F32)
        nc.sync.dma_start(sc_sb[:], scale[0:1, :].to_broadcast((P, C)))
        for h in range(NH):
            wdh = wd_pool.tile([P, NKF * P], F32R, tag="wdh")
            nc.sync.dma_start(wdh[:], wd_v[:, :, h * P:(h + 1) * P])
            for off, sz in CH:
                csl = slice(off, off + sz)
                py_t = psB.tile([P, MAX_CHUNK], F32, tag="py")
                py = py_t[:, :sz]
                for k in range(NKF):
                    nc.tensor.matmul(py, wdh[:, k * P:(k + 1) * P],
                                     at_sb[k][:, csl],
                                     start=(k == 0), stop=(k == NKF - 1))
                yo_t = yo_pool.tile([P, MAX_CHUNK], F32, tag="yo")
                yo = yo_t[:, :sz]
                nc.vector.tensor_mul(yo, py, sc_sb[:, csl])
                nc.sync.dma_start(yt[h * P:(h + 1) * P, off:off + sz], yo)


def _route(hidden: np.ndarray, gate_w: np.ndarray):
    """Host router: returns (idx [T, K], w [T, K] renormalized fp32)."""
    logits = hidden.astype(np.float32) @ gate_w.astype(np.float32)
    m = logits.max(axis=-1, keepdims=True)
    e = np.exp((logits - m).astype(np.float32))
    p = e / e.sum(axis=-1, keepdims=True)
    # top-2, ties -> lower index (match jax.lax.top_k)
    order = np.argsort(-p, axis=-1, kind="stable")
    idx = order[:, :TOP_K]
    topw = np.take_along_axis(p, idx, axis=-1)
    topw = topw / topw.sum(axis=-1, keepdims=True)
    return idx, topw.astype(np.float32)


def kernel(hidden_states, gate_w, gate_proj_w, up_proj_w, down_proj_w):
    B, S, Hx = hidden_states.shape
    T = B * S
    hidden = np.ascontiguousarray(
        np.asarray(hidden_states, dtype=np.float32).reshape(T, Hx))

    idx, topw = _route(hidden, np.asarray(gate_w))

    # Per-expert token lists
    rows, wts = [], []
    for e in range(E):
        mask = (idx == e)
        r = np.nonzero(mask.any(axis=-1))[0]
        rows.append(r)
        wts.append(topw[r, np.argmax(idx[r] == e, axis=-1)])
    maxn = max(len(r) for r in rows)
    C = max(MIN_CHUNK, ((maxn + 7) // 8) * 8)

    if C not in _cache:
        _cache[C] = build_moe_program(C)
    nc = _cache[C]

    gate_proj_w = np.asarray(gate_proj_w, dtype=np.float32)
    up_proj_w = np.asarray(up_proj_w, dtype=np.float32)
    down_proj_w = np.asarray(down_proj_w, dtype=np.float32)

    in_maps = []
    for e in range(E):
        r = rows[e]
        xt = np.zeros((Hx, C), dtype=np.float32)
        xt[:, :len(r)] = hidden[r].T
        sc = np.zeros((1, C), dtype=np.float32)
        sc[0, :len(r)] = wts[e]
        in_maps.append({
            "xt": xt,
            "wg": np.ascontiguousarray(gate_proj_w[e]),
            "wu": np.ascontiguousarray(up_proj_w[e]),
            "wd": np.ascontiguousarray(down_proj_w[e]),
            "scale": sc,
        })

    global _last_in_maps, _last_rows
    _last_in_maps = in_maps
    _last_rows = rows
    res = run_bass_kernel_spmd(nc, in_maps, core_ids=list(range(E)))

    out = np.zeros((T, Hx), dtype=np.float32)
    for e in range(E):
        r = rows[e]
        out[r] += res.results[e]["yt"][:, :len(r)].T
    return out.reshape(B, S, Hx)

